# revision 1
# baseline (speedup 1.0000x reference)
"""ONI-Norm TRN2 kernel v4: group-interleaved emission to keep the PE warm.

Same math and op idioms as v1 (see kernel_good.py); only the program order
changes: the serial Newton-Schulz chain of group 0 is interleaved with
group 1's Gram slices, and NS of group 1 with group 0's projection, so the
tensor engine never idles >3.4us (avoids HAM re-throttle to 1.2 GHz).
"""

import math
from contextlib import ExitStack

import numpy as np

import concourse.bacc as bacc
import concourse.mybir as mybir
from concourse.bass import ds, ts, MemorySpace
from concourse.bass_isa import ReduceOp
from concourse.bass_utils import run_bass_kernel_spmd
from concourse.masks import make_identity
from concourse.tile import TileContext

P = 128
K = 18432
G_TOTAL = 16
N_CORES = 8
G_PER_CORE = G_TOTAL // N_CORES
ROWS_PER_CORE = G_PER_CORE * P
T_NS = 5
EPS = 1e-5
CHUNK = 2048
N_CHUNKS = K // CHUNK
SUB = 512
SUB_PER_CHUNK = CHUNK // SUB
N_SLICES = N_CHUNKS * SUB_PER_CHUNK   # 36 per group
F32 = mybir.dt.float32


def build_nc():
    nc = bacc.Bacc("TRN2", target_bir_lowering=False)
    x = nc.dram_tensor("x", [ROWS_PER_CORE, K], F32, kind="ExternalInput")
    y = nc.dram_tensor("y", [ROWS_PER_CORE, K], F32, kind="ExternalOutput")

    with TileContext(nc) as tc, ExitStack() as ctx:
        consts = ctx.enter_context(tc.tile_pool(name="consts", bufs=1))
        identity = consts.tile([P, P], F32)
        make_identity(nc, identity)
        eye_15 = consts.tile([P, P], F32)
        nc.vector.tensor_scalar_mul(eye_15, identity, 1.5)
        eps_eye = consts.tile([P, P], F32)
        nc.vector.tensor_scalar_mul(eps_eye, identity, EPS)
        ones = consts.tile([P, P], F32)
        nc.any.memset(ones, 1.0)

        zpool = ctx.enter_context(tc.tile_pool(name="z", bufs=G_PER_CORE * N_CHUNKS))
        ztp = ctx.enter_context(tc.tile_pool(name="zt", bufs=3))
        outp = ctx.enter_context(tc.tile_pool(name="out", bufs=3))
        nsp = ctx.enter_context(tc.tile_pool(name="ns", bufs=2))
        vecp = ctx.enter_context(tc.tile_pool(name="vec", bufs=2))
        ps_S = ctx.enter_context(tc.tile_pool(name="psS", bufs=2, space=MemorySpace.PSUM))
        ps_big = ctx.enter_context(tc.tile_pool(name="psB", bufs=3, space=MemorySpace.PSUM))
        ps_ns = ctx.enter_context(tc.tile_pool(name="psN", bufs=2, space=MemorySpace.PSUM))
        ps_vec = ctx.enter_context(tc.tile_pool(name="psV", bufs=1, space=MemorySpace.PSUM))

        st = [dict() for _ in range(G_PER_CORE)]

        def emit_loads(g):
            s = st[g]
            s["zs"] = []
            s["rsum_parts"] = vecp.tile([P, N_CHUNKS], F32, name=f"rsp{g}")
            for c in range(N_CHUNKS):
                z = zpool.tile([P, CHUNK], F32, tag="z", name=f"z{g}_{c}")
                if g == 0 and c == 0:
                    # smaller first transfers so the PE starts sooner
                    for t4 in range(SUB_PER_CHUNK):
                        nc.sync.dma_start(
                            z[:, ts(t4, SUB)],
                            x[ds(g * P, P), ds(c * CHUNK + t4 * SUB, SUB)],
                        )
                else:
                    nc.sync.dma_start(z, x[ds(g * P, P), ts(c, CHUNK)])
                nc.vector.tensor_reduce(
                    s["rsum_parts"][:, ds(c, 1)], z,
                    mybir.AxisListType.X, mybir.AluOpType.add,
                )
                s["zs"].append(z)

        def emit_gram_T(g, si):
            s = st[g]
            c, t = divmod(si, SUB_PER_CHUNK)
            tp = ps_big.tile([P, SUB], F32, tag="big", name=f"tp{g}_{si}")
            for b in range(SUB // P):
                nc.tensor.transpose(
                    tp[:, ts(b, P)],
                    s["zs"][c][:, ds(t * SUB + b * P, P)],
                    identity,
                )
            zt = ztp.tile([P, SUB], F32, tag="zt", name=f"zt{g}_{si}")
            # ACT copy: DVE is the per-chunk rate limiter during Gram
            # (row-sum reduce runs at 1x); ACT is idle here.
            nc.scalar.copy(zt, tp)
            s.setdefault("zt_pend", {})[si] = zt

        def emit_gram_M(g, si):
            s = st[g]
            if si == 0:
                s["S_ps"] = ps_S.tile([P, P], F32, tag="S", name=f"Sps{g}")
            zt = s["zt_pend"].pop(si)
            for b in range(SUB // P):
                nc.tensor.matmul(
                    s["S_ps"], zt[:, ts(b, P)], zt[:, ts(b, P)],
                    start=(si == 0 and b == 0), stop=False,
                )

        def emit_gram_slice(g, si, pipelined=True):
            # transposes of slice si, then matmuls of slice si-1 (1-slice lag)
            emit_gram_T(g, si)
            if si > 0:
                emit_gram_M(g, si - 1)
            if si == N_SLICES - 1:
                emit_gram_M(g, si)

        def emit_mean_chain(g):
            s = st[g]
            rsum = vecp.tile([P, 1], F32, name=f"rs{g}")
            nc.vector.tensor_reduce(
                rsum, s["rsum_parts"], mybir.AxisListType.X, mybir.AluOpType.add
            )
            mean = vecp.tile([P, 1], F32, name=f"mean{g}")
            nc.vector.tensor_scalar_mul(mean, rsum, 1.0 / K)
            s["mean"] = mean
            m12 = vecp.tile([P, 1], F32, name=f"m12{g}")
            nc.vector.tensor_scalar_mul(m12, rsum, math.sqrt(K / P) / K)
            Mm = vecp.tile([P, P], F32, name=f"Mm{g}")
            nc.vector.tensor_scalar_mul(Mm, ones, m12)
            M_ps = ps_vec.tile([P, P], F32, tag="v", name=f"Mps{g}")
            nc.tensor.matmul(M_ps, Mm, identity, start=True, stop=True)
            M128a = vecp.tile([P, P], F32, name=f"Ma{g}")
            nc.vector.tensor_copy(M128a, M_ps)
            M128b = vecp.tile([P, P], F32, name=f"Mb{g}")
            nc.vector.tensor_scalar_mul(M128b, M128a, -1.0)
            nc.tensor.matmul(s["S_ps"], M128a, M128b, start=False, stop=True)

            S = nsp.tile([P, P], F32, tag="S", name=f"S{g}")
            nc.vector.tensor_copy(S, s["S_ps"])
            nc.vector.tensor_add(S, S, eps_eye)
            S2 = nsp.tile([P, P], F32, tag="S2", name=f"S2_{g}")
            frob2 = vecp.tile([P, 1], F32, name=f"fr{g}")
            nc.scalar.activation(
                S2, S, mybir.ActivationFunctionType.Square, accum_out=frob2
            )
            nc.gpsimd.partition_all_reduce(frob2, frob2, P, ReduceOp.add)
            nu = vecp.tile([P, 1], F32, name=f"nu{g}")
            nc.scalar.sqrt(nu, frob2)
            inv_nu = vecp.tile([P, 1], F32, name=f"inu{g}")
            nc.vector.reciprocal(inv_nu, nu)
            oscale = vecp.tile([P, 1], F32, name=f"osc{g}")
            nc.scalar.sqrt(oscale, inv_nu)
            s["oscale"] = oscale
            Sn = nsp.tile([P, P], F32, tag="Sn", name=f"Sn{g}")
            nc.vector.tensor_scalar_mul(Sn, S, inv_nu)
            S_half = nsp.tile([P, P], F32, tag="Sh", name=f"Sh{g}")
            nc.vector.tensor_scalar_mul(S_half, Sn, 0.5)
            s["S_half"] = S_half
            B = nsp.tile([P, P], F32, tag=f"B{g}", name=f"B0_{g}")
            nc.vector.tensor_sub(B, eye_15, S_half)
            s["B"] = B

        def emit_ns_step(g, it, sub):
            # one PE matmul of the NS chain + its trailing DVE op(s)
            s = st[g]
            if sub == 0:
                bb_ps = ps_ns.tile([P, P], F32, tag="ns", name=f"bb{g}_{it}")
                nc.tensor.matmul(bb_ps, s["B"], s["B"], start=True, stop=True)
                BB = nsp.tile([P, P], F32, tag=f"BB{g}", name=f"BB{g}_{it}")
                nc.vector.tensor_copy(BB, bb_ps)
                s["BB"] = BB
            elif sub == 1:
                b3_ps = ps_ns.tile([P, P], F32, tag="ns", name=f"b3{g}_{it}")
                nc.tensor.matmul(b3_ps, s["BB"], s["B"], start=True, stop=True)
                B3 = nsp.tile([P, P], F32, tag=f"B3{g}", name=f"B3_{g}_{it}")
                nc.vector.tensor_copy(B3, b3_ps)
                s["B3"] = B3
            else:
                p_ps = ps_ns.tile([P, P], F32, tag="ns", name=f"pp{g}_{it}")
                nc.tensor.matmul(p_ps, s["B3"], s["S_half"], start=True, stop=True)
                Bn = nsp.tile([P, P], F32, tag=f"Bn{g}", name=f"Bn{g}_{it}")
                nc.vector.tensor_scalar_mul(Bn, s["B"], 1.5)
                nc.vector.tensor_sub(Bn, Bn, p_ps)
                s["B"] = Bn

        def emit_cbias(g):
            s = st[g]
            c_ps = ps_vec.tile([P, 1], F32, tag="v", name=f"cps{g}")
            nc.tensor.matmul(c_ps, s["B"], s["mean"], start=True, stop=True)
            negos = vecp.tile([P, 1], F32, name=f"ng{g}")
            nc.vector.tensor_scalar_mul(negos, s["oscale"], -1.0)
            bias = vecp.tile([P, 1], F32, name=f"bi{g}")
            nc.vector.tensor_mul(bias, negos, c_ps)
            s["bias"] = bias

        def emit_proj_slice(g, si):
            s = st[g]
            c, t = divmod(si, SUB_PER_CHUNK)
            if t == 0:
                s["out_t"] = outp.tile([P, CHUNK], F32, tag="out", name=f"o{g}_{c}")
            pr = ps_big.tile([P, SUB], F32, tag="big", name=f"pr{g}_{si}")
            nc.tensor.matmul(
                pr, s["B"], s["zs"][c][:, ts(t, SUB)], start=True, stop=True
            )
            nc.scalar.activation(
                s["out_t"][:, ts(t, SUB)], pr,
                mybir.ActivationFunctionType.Identity,
                bias=s["bias"], scale=s["oscale"],
            )
            if t == SUB_PER_CHUNK - 1:
                nc.sync.dma_start(y[ds(g * P, P), ts(c, CHUNK)], s["out_t"])

        # ---------------- emission schedule ----------------
        emit_loads(0)
        emit_loads(1)
        for si in range(N_SLICES):
            emit_gram_slice(0, si)
        emit_mean_chain(0)

        # NS(g0) interleaved with Gram(g1): 12 NS matmuls, 36 gram slices
        g1_si = 0
        for it in range(T_NS - 1):
            for sub in range(3):
                emit_ns_step(0, it, sub)
                for _ in range(2):
                    if g1_si < N_SLICES:
                        emit_gram_slice(1, g1_si)
                        g1_si += 1
        while g1_si < N_SLICES:
            emit_gram_slice(1, g1_si)
            g1_si += 1
        emit_mean_chain(1)
        emit_cbias(0)

        # NS(g1) interleaved with proj(g0): 12 NS matmuls, 36 proj slices
        p0_si = 0
        for it in range(T_NS - 1):
            for sub in range(3):
                emit_ns_step(1, it, sub)
                for _ in range(2):
                    if p0_si < N_SLICES:
                        emit_proj_slice(0, p0_si)
                        p0_si += 1
        while p0_si < N_SLICES:
            emit_proj_slice(0, p0_si)
            p0_si += 1
        emit_cbias(1)
        for si in range(N_SLICES):
            emit_proj_slice(1, si)

    nc.finalize()
    return nc


_NC_CACHE = None


def _get_nc():
    global _NC_CACHE
    if _NC_CACHE is None:
        _NC_CACHE = build_nc()
    return _NC_CACHE


def kernel(weight, _trace=False):
    w = np.ascontiguousarray(np.asarray(weight, dtype=np.float32))
    assert w.shape == (G_TOTAL * P, K), w.shape
    nc = _get_nc()
    in_maps = [
        {"x": np.ascontiguousarray(w[core * ROWS_PER_CORE:(core + 1) * ROWS_PER_CORE])}
        for core in range(N_CORES)
    ]
    res = run_bass_kernel_spmd(
        nc, in_maps, core_ids=list(range(N_CORES)), trace=_trace
    )
    out = np.concatenate([r["y"] for r in res.results], axis=0)
    if _trace:
        return out, res
    return out



# revision 10
# speedup vs baseline: 1.4278x; 1.4278x over previous
"""ONI-Norm TRN2 kernel v5: bf16 datapath.

The fp32 baseline was PE-bound (~160us of 4-cycle/row fp32 matmuls).
This version loads Z as bf16 via casting SWDGE DMA, runs transposes /
Gram / Newton-Schulz / projection in bf16 (1 cycle/row, FWL weight
loads), accumulating in fp32 PSUM. PE drops to ~60us, and the kernel
becomes HBM-DMA-bound (18.9MB in + 18.9MB out per core ~ 95us).
Row-sums run on GpSimd, PSUM->SBUF moves alternate ACT/DVE, outputs
stream per-chunk as soon as each group's projection completes.

Numerics: all-bf16 pipeline measured at ~7e-3 max rel err vs fp32
reference (tolerance 2e-2); fp32 mean/frob/scale chain keeps the
normalization exact.
"""

import math
from contextlib import ExitStack

import numpy as np

import concourse.bacc as bacc
import concourse.mybir as mybir
from concourse.bass import ds, ts, MemorySpace
from concourse.bass_isa import ReduceOp
from concourse.bass_utils import run_bass_kernel_spmd
from concourse.masks import make_identity
from concourse.tile import TileContext

P = 128
K = 18432
G_TOTAL = 16
N_CORES = 8
G_PER_CORE = G_TOTAL // N_CORES
ROWS_PER_CORE = G_PER_CORE * P
T_NS = 5
EPS = 1e-5
CHUNK = 2048
N_CHUNKS = K // CHUNK
SUB = 512
SUB_PER_CHUNK = CHUNK // SUB
N_SLICES = N_CHUNKS * SUB_PER_CHUNK   # 36 per group
F32 = mybir.dt.float32
BF16 = mybir.dt.bfloat16
AX = mybir.AxisListType.X
ADD = mybir.AluOpType.add
MULT = mybir.AluOpType.mult
SUBTRACT = mybir.AluOpType.subtract
IDENT = mybir.ActivationFunctionType.Identity


def build_nc():
    nc = bacc.Bacc("TRN2", target_bir_lowering=False)
    x = nc.dram_tensor("x", [ROWS_PER_CORE, K], F32, kind="ExternalInput")
    y = nc.dram_tensor("y", [ROWS_PER_CORE, K], F32, kind="ExternalOutput")

    with TileContext(nc) as tc, ExitStack() as ctx:
        consts = ctx.enter_context(tc.tile_pool(name="consts", bufs=1))
        identity = consts.tile([P, P], BF16)
        make_identity(nc, identity)  # gpsimd; emitted before load DMAs
        eye_15 = consts.tile([P, P], BF16)
        nc.vector.tensor_scalar_mul(eye_15, identity, 1.5)
        eps_eye = consts.tile([P, P], F32)
        nc.vector.tensor_scalar_mul(eps_eye, identity, EPS)
        ones = consts.tile([P, P], BF16)
        nc.vector.memset(ones, 1.0)

        zpool = ctx.enter_context(tc.tile_pool(name="z", bufs=G_PER_CORE * N_CHUNKS))
        ztp = ctx.enter_context(tc.tile_pool(name="zt", bufs=4))
        outp = ctx.enter_context(tc.tile_pool(name="out", bufs=4))
        nsp = ctx.enter_context(tc.tile_pool(name="ns", bufs=1))
        vecp = ctx.enter_context(tc.tile_pool(name="vec", bufs=1))
        ps_S = ctx.enter_context(tc.tile_pool(name="psS", bufs=2, space=MemorySpace.PSUM))
        ps_big = ctx.enter_context(tc.tile_pool(name="psB", bufs=4, space=MemorySpace.PSUM))
        ps_ns = ctx.enter_context(tc.tile_pool(name="psN", bufs=2, space=MemorySpace.PSUM))

        st = [dict() for _ in range(G_PER_CORE)]
        move_ctr = [0]  # alternate ACT/DVE for PSUM->SBUF moves

        def psum_move(dst, src, bias=None, scale=None):
            move_ctr[0] += 1
            if bias is not None:
                # projection output: out = src*scale + bias
                if move_ctr[0] % 2 == 0:
                    nc.scalar.activation(dst, src, IDENT, bias=bias, scale=scale)
                else:
                    nc.vector.tensor_scalar(dst, src, scale, bias, MULT, ADD)
            else:
                if move_ctr[0] % 2 == 0:
                    nc.scalar.copy(dst, src)
                else:
                    nc.vector.tensor_copy(dst, src)

        def emit_load(g, c):
            s = st[g]
            if c == 0:
                s["zs"] = []
            z = zpool.tile([P, CHUNK], BF16, tag="z", name=f"z{g}_{c}")
            nc.gpsimd.dma_start(z, x[ds(g * P, P), ts(c, CHUNK)])  # SWDGE cast f32->bf16
            s["zs"].append(z)

        def emit_gram_T(g, si):
            s = st[g]
            c, t = divmod(si, SUB_PER_CHUNK)
            tp = ps_big.tile([P, SUB], BF16, tag="big", name=f"tp{g}_{si}")
            for b in range(SUB // P):
                nc.tensor.transpose(
                    tp[:, ts(b, P)],
                    s["zs"][c][:, ds(t * SUB + b * P, P)],
                    identity,
                )
            zt = ztp.tile([P, SUB], BF16, tag="zt", name=f"zt{g}_{si}")
            psum_move(zt, tp)
            s.setdefault("zt_pend", {})[si] = zt

        def emit_gram_M(g, si):
            s = st[g]
            if si == 0:
                # column 128 accumulates the row-sum (Z @ ones) on the PE
                s["S_ps"] = ps_S.tile([P, P + 1], F32, tag="S", name=f"Sps{g}")
            zt = s["zt_pend"].pop(si)
            first = si == 0
            last = si == N_SLICES - 1
            for b in range(SUB // P):
                nc.tensor.matmul(
                    s["S_ps"][:, ds(0, P)], zt[:, ts(b, P)], zt[:, ts(b, P)],
                    start=(first and b == 0), stop=False,
                )
                nc.tensor.matmul(
                    s["S_ps"][:, ds(P, 1)], zt[:, ts(b, P)], ones[:, ds(0, 1)],
                    start=(first and b == 0), stop=(last and b == SUB // P - 1),
                )

        def emit_gram_slice(g, si):
            # transposes of slice si, matmuls of slice si-2 (2-slice lag so the
            # PSUM->SBUF move of a slice hides under the next slice's PE work)
            emit_gram_T(g, si)
            if si >= 2:
                emit_gram_M(g, si - 2)
            if si == N_SLICES - 1:
                emit_gram_M(g, si - 1)
                emit_gram_M(g, si)

        def emit_mean_chain(g):
            s = st[g]
            rsum = s["S_ps"][:, ds(P, 1)]  # accumulated on PE during gram
            mean_bf = vecp.tile([P, 1], BF16, name=f"mean{g}")
            nc.vector.tensor_scalar_mul(mean_bf, rsum, 1.0 / K)
            s["mean_bf"] = mean_bf
            m12 = vecp.tile([P, 1], F32, name=f"m12{g}")
            nc.vector.tensor_scalar_mul(m12, rsum, math.sqrt(K / P) / K)
            Mm = vecp.tile([P, P], BF16, name=f"Mm{g}")
            nc.vector.tensor_scalar_mul(Mm, ones, m12)
            M_ps = ps_ns.tile([P, P], F32, tag="ns", name=f"Mps{g}")
            nc.tensor.matmul(M_ps, Mm, identity, start=True, stop=True)
            M128a = vecp.tile([P, P], BF16, name=f"Ma{g}")
            nc.vector.tensor_copy(M128a, M_ps)
            M128b = vecp.tile([P, P], BF16, name=f"Mb{g}")
            nc.vector.tensor_scalar_mul(M128b, M128a, -1.0)
            nc.tensor.matmul(s["S_ps"][:, ds(0, P)], M128a, M128b, start=False, stop=True)

            # S = S_ps + eps*I  (fp32, one DVE op)
            S = nsp.tile([P, P], F32, name=f"S{g}")
            nc.vector.tensor_add(S, s["S_ps"][:, ds(0, P)], eps_eye)
            S2 = nsp.tile([P, P], F32, name=f"S2_{g}")
            frob2 = vecp.tile([P, 1], F32, name=f"fr{g}")
            nc.scalar.activation(
                S2, S, mybir.ActivationFunctionType.Square, accum_out=frob2
            )
            nc.gpsimd.partition_all_reduce(frob2, frob2, P, ReduceOp.add)
            nu = vecp.tile([P, 1], F32, name=f"nu{g}")
            nc.scalar.sqrt(nu, frob2)
            inv_nu = vecp.tile([P, 1], F32, name=f"inu{g}")
            nc.vector.reciprocal(inv_nu, nu)
            oscale = vecp.tile([P, 1], F32, name=f"osc{g}")
            nc.scalar.sqrt(oscale, inv_nu)
            s["oscale"] = oscale
            half_inv = vecp.tile([P, 1], F32, name=f"hinu{g}")
            nc.vector.tensor_scalar_mul(half_inv, inv_nu, 0.5)
            S_half = nsp.tile([P, P], BF16, name=f"Sh{g}")
            nc.vector.tensor_scalar_mul(S_half, S, half_inv)
            s["S_half"] = S_half
            B = nsp.tile([P, P], BF16, name=f"B0_{g}", tag=f"B{g}", bufs=2)
            nc.vector.tensor_sub(B, eye_15, S_half)
            s["B"] = B

        def emit_ns_step(g, it, sub):
            # one PE matmul of the NS chain + its trailing DVE op
            s = st[g]
            if sub == 0:
                bb_ps = ps_ns.tile([P, P], F32, tag="ns", name=f"bb{g}_{it}")
                nc.tensor.matmul(bb_ps, s["B"], s["B"], start=True, stop=True)
                BB = nsp.tile([P, P], BF16, name=f"BB{g}_{it}", tag=f"BB{g}", bufs=2)
                nc.vector.tensor_copy(BB, bb_ps)
                s["BB"] = BB
            elif sub == 1:
                b3_ps = ps_ns.tile([P, P], F32, tag="ns", name=f"b3{g}_{it}")
                nc.tensor.matmul(b3_ps, s["BB"], s["B"], start=True, stop=True)
                B3 = nsp.tile([P, P], BF16, name=f"B3_{g}_{it}", tag=f"B3{g}", bufs=2)
                nc.vector.tensor_copy(B3, b3_ps)
                s["B3"] = B3
            else:
                p_ps = ps_ns.tile([P, P], F32, tag="ns", name=f"pp{g}_{it}")
                nc.tensor.matmul(p_ps, s["B3"], s["S_half"], start=True, stop=True)
                Bn = nsp.tile([P, P], BF16, name=f"Bn{g}_{it}", tag=f"B{g}", bufs=2)
                # Bn = 1.5*B - p_ps in one DVE op
                nc.vector.scalar_tensor_tensor(Bn, s["B"], 1.5, p_ps, MULT, SUBTRACT)
                s["B"] = Bn

        def emit_cbias(g):
            s = st[g]
            c_ps = ps_ns.tile([P, 1], F32, tag="ns", name=f"cps{g}")
            nc.tensor.matmul(c_ps, s["B"], s["mean_bf"], start=True, stop=True)
            negos = vecp.tile([P, 1], F32, name=f"ng{g}")
            nc.vector.tensor_scalar_mul(negos, s["oscale"], -1.0)
            bias = vecp.tile([P, 1], F32, name=f"bi{g}")
            nc.vector.tensor_mul(bias, negos, c_ps)
            s["bias"] = bias

        def emit_proj_slice(g, si):
            s = st[g]
            c, t = divmod(si, SUB_PER_CHUNK)
            if t == 0:
                s["out_t"] = outp.tile([P, CHUNK], F32, tag="out", name=f"o{g}_{c}")
            pr = ps_big.tile([P, SUB], F32, tag="big", name=f"pr{g}_{si}")
            nc.tensor.matmul(
                pr, s["B"], s["zs"][c][:, ts(t, SUB)], start=True, stop=True
            )
            psum_move(s["out_t"][:, ts(t, SUB)], pr,
                      bias=s["bias"], scale=s["oscale"])
            if t == SUB_PER_CHUNK - 1:
                nc.sync.dma_start(y[ds(g * P, P), ts(c, CHUNK)], s["out_t"])

        # ---------------- emission schedule ----------------
        # all load DMAs first so SWDGE descriptor generation stays ahead of
        # the SDMA engines; per-chunk row-sum reduces follow on the same
        # (gpsimd) queue and wait for their chunk's arrival.
        for g in range(G_PER_CORE):
            for c in range(N_CHUNKS):
                emit_load(g, c)

        for si in range(N_SLICES):
            emit_gram_slice(0, si)
        emit_mean_chain(0)

        # NS(g0) interleaved with Gram(g1): 15 NS matmuls, 36 gram slices.
        # g1's chunks are still streaming in, so gram(1) is DMA-paced here.
        g1_si = 0
        for it in range(T_NS - 1):
            for sub in range(3):
                emit_ns_step(0, it, sub)
                for _ in range(2):
                    if g1_si < N_SLICES:
                        emit_gram_slice(1, g1_si)
                        g1_si += 1
        emit_cbias(0)

        # proj(g0) streams output early, interleaved with the tail of
        # gram(g1) and then NS(g1).
        p0_si = 0
        while g1_si < N_SLICES:
            emit_gram_slice(1, g1_si)
            g1_si += 1
            if p0_si < N_SLICES:
                emit_proj_slice(0, p0_si)
                p0_si += 1
        emit_mean_chain(1)
        for it in range(T_NS - 1):
            for sub in range(3):
                emit_ns_step(1, it, sub)
                for _ in range(2):
                    if p0_si < N_SLICES:
                        emit_proj_slice(0, p0_si)
                        p0_si += 1
        while p0_si < N_SLICES:
            emit_proj_slice(0, p0_si)
            p0_si += 1
        emit_cbias(1)
        for si in range(N_SLICES):
            emit_proj_slice(1, si)

    nc.finalize()
    return nc


_NC_CACHE = None


def _get_nc():
    global _NC_CACHE
    if _NC_CACHE is None:
        _NC_CACHE = build_nc()
    return _NC_CACHE


def kernel(weight, _trace=False):
    w = np.ascontiguousarray(np.asarray(weight, dtype=np.float32))
    assert w.shape == (G_TOTAL * P, K), w.shape
    nc = _get_nc()
    in_maps = [
        {"x": np.ascontiguousarray(w[core * ROWS_PER_CORE:(core + 1) * ROWS_PER_CORE])}
        for core in range(N_CORES)
    ]
    res = run_bass_kernel_spmd(
        nc, in_maps, core_ids=list(range(N_CORES)), trace=_trace
    )
    out = np.concatenate([r["y"] for r in res.results], axis=0)
    if _trace:
        return out, res
    return out
